# revision 1
# baseline (speedup 1.0000x reference)
"""Causal self-attention (query-axis softmax) for Trainium2, 8 NeuronCores.

Sharding: 8 cores = 4 batches x 2 half-head-groups. Core c handles batch
c//2 and heads (c%2)*6 .. (c%2)*6+5. Each core computes its heads' full
attention plus its partial output projection; the host sums the two
partials per batch and adds b_proj.

Layout strategy per core (T=2048, C=768, 6 heads, hd=64):
  - host passes x[b].T so the QKV contraction dim (C) lands on SBUF
    partitions without any on-chip transpose.
  - Q,K are produced transposed ([head_d, t]) so S^T = K Q^T tiles have
    softmax's query axis on the free dimension; V is produced in [t, d].
  - softmax over q (free axis): no max-subtraction needed (logits are
    O(1) by construction), exp+rowsum fused on ScalarE via accum_out,
    normalization folded into V rows (scale V[k,:] by 1/denom[k]).
  - causal mask: ragged chunk bounds skip fully-masked blocks; diagonal
    128x128 blocks get a precomputed triangular -30000 add.
  - all matmuls run as float32r (full PE rate at moving dim >= 256,
    ~1e-4 relative error).
"""

import os
import sys

sys.path.insert(0, "/opt/trn_rl_repo")

import numpy as np

import concourse.bass as bass
import concourse.mybir as mybir
import concourse.tile as tile
from concourse.bass_utils import run_bass_kernel_spmd

FP32 = mybir.dt.float32
FP32R = mybir.dt.float32r
BF16 = mybir.dt.bfloat16

B, T, C, H = 4, 2048, 768, 12
D = 64                  # head dim
NCORES = 8
HPC = H * B // NCORES   # heads per core = 6
E = HPC * D             # qkv slice width per core = 384
CT = C // 128           # c tiles = 6
ET = E // 128           # e tiles = 3
TT = T // 128           # t tiles = 16
QCH = 512               # q chunk
NQC = T // QCH          # q chunks = 4
MASKV = -30000.0
SCALE = 1.0 / 8.0       # 1/sqrt(hd)


def _split_sync_waits(nc):
    """This container's walrus encodes at most one sync wait per
    instruction for several instruction structs; hoist extra waits onto
    same-engine nops placed immediately before the instruction."""
    for f in nc.m.functions:
        for bb in f.blocks:
            new_insts = []
            for inst in bb.instructions:
                si = inst.sync_info
                waits = list(si.on_wait) if si is not None and si.on_wait else []
                if len(waits) > 1:
                    for w in waits[:-1]:
                        nop = mybir.InstNoOp(
                            name=nc.get_next_instruction_name(),
                            engine=inst.engine,
                            sync_info=mybir.SyncInfo(on_wait=[w], on_update=[]),
                            bass_nofuse=True,
                        )
                        nc.register_instruction(nop)
                        new_insts.append(nop)
                    inst.sync_info = mybir.SyncInfo(
                        on_wait=[waits[-1]], on_update=list(si.on_update or [])
                    )
                new_insts.append(inst)
            bb.instructions[:] = new_insts


def _build(pair_bf16_av: bool):
    nc = bass.Bass("TRN2")
    xT = nc.dram_tensor("xT", [C, T], FP32, kind="ExternalInput")
    wq = nc.dram_tensor("wq", [C, E], FP32, kind="ExternalInput")
    wk = nc.dram_tensor("wk", [C, E], FP32, kind="ExternalInput")
    wv = nc.dram_tensor("wv", [C, E], FP32, kind="ExternalInput")
    bq = nc.dram_tensor("bq", [E], FP32, kind="ExternalInput")
    bk = nc.dram_tensor("bk", [E], FP32, kind="ExternalInput")
    bv = nc.dram_tensor("bv", [E], FP32, kind="ExternalInput")
    wp = nc.dram_tensor("wp", [E, C], FP32, kind="ExternalInput")
    mask = nc.dram_tensor("mask", [128, 128], FP32, kind="ExternalInput")
    out = nc.dram_tensor("out", [T, C], FP32, kind="ExternalOutput")

    av_dt = BF16 if pair_bf16_av else FP32R

    with tile.TileContext(nc) as tc:
        with (
            tc.tile_pool(name="wts", bufs=1) as wts,
            tc.tile_pool(name="xp", bufs=2) as xp,
            tc.tile_pool(name="big", bufs=1) as big,
            tc.tile_pool(name="atp", bufs=3) as atp,
            tc.tile_pool(name="sm", bufs=4) as sm,
            tc.tile_pool(name="op", bufs=3) as op,
            tc.tile_pool(name="ps", bufs=4, space="PSUM") as ps,
            tc.tile_pool(name="psy", bufs=1, space="PSUM") as psy,
        ):
            # ---- constant loads ----
            wq_sb = wts.tile([128, CT, E], FP32R)
            wk_sb = wts.tile([128, CT, E], FP32R)
            wv_sb = wts.tile([128, CT, E], FP32R)
            nc.sync.dma_start(out=wq_sb, in_=wq.rearrange("(ct p) e -> p ct e", p=128).bitcast(FP32R))
            nc.sync.dma_start(out=wk_sb, in_=wk.rearrange("(ct p) e -> p ct e", p=128).bitcast(FP32R))
            nc.sync.dma_start(out=wv_sb, in_=wv.rearrange("(ct p) e -> p ct e", p=128).bitcast(FP32R))
            wp_sb = wts.tile([128, ET, C], FP32R)
            nc.sync.dma_start(out=wp_sb, in_=wp.rearrange("(et p) c -> p et c", p=128).bitcast(FP32R))
            bq_sb = wts.tile([128, ET], FP32)
            bk_sb = wts.tile([128, ET], FP32)
            nc.sync.dma_start(out=bq_sb, in_=bq.rearrange("(et p) -> p et", p=128))
            nc.sync.dma_start(out=bk_sb, in_=bk.rearrange("(et p) -> p et", p=128))
            bv_sb = wts.tile([128, E], FP32)
            nc.sync.dma_start(out=bv_sb, in_=bv[None, :].to_broadcast((128, E)))
            mask_sb = wts.tile([128, 128], FP32)
            nc.sync.dma_start(out=mask_sb, in_=mask[:])

            qt_sb = big.tile([128, ET, T], FP32R)   # [d-in-pair, pair, t]
            kt_sb = big.tile([128, ET, T], FP32R)
            v_sb = big.tile([128, TT, E], av_dt)    # [t-in-tile, ttile, (head,d)]
            y_sb = big.tile([128, ET, T], FP32R)    # [hd-in-pair, pair, t]

            # ---- QKV ----
            for tci in range(NQC):
                xt = xp.tile([128, CT, QCH], FP32R, tag="xt", bufs=2)
                nc.sync.dma_start(
                    out=xt,
                    in_=xT[:, tci * QCH:(tci + 1) * QCH]
                    .rearrange("(ct p) t -> p ct t", p=128).bitcast(FP32R),
                )
                for et in range(ET):
                    pq = ps.tile([128, QCH], FP32, tag="ps", bufs=4)
                    for ct in range(CT):
                        nc.tensor.matmul(
                            pq, wq_sb[:, ct, et * 128:(et + 1) * 128], xt[:, ct, :],
                            start=(ct == 0), stop=(ct == CT - 1),
                        )
                    nc.vector.tensor_scalar_add(
                        qt_sb[:, et, tci * QCH:(tci + 1) * QCH], pq, bq_sb[:, et:et + 1]
                    )
                    pk = ps.tile([128, QCH], FP32, tag="ps", bufs=4)
                    for ct in range(CT):
                        nc.tensor.matmul(
                            pk, wk_sb[:, ct, et * 128:(et + 1) * 128], xt[:, ct, :],
                            start=(ct == 0), stop=(ct == CT - 1),
                        )
                    nc.vector.tensor_scalar_add(
                        kt_sb[:, et, tci * QCH:(tci + 1) * QCH], pk, bk_sb[:, et:et + 1]
                    )
                for ttl in range(4):
                    tt = tci * 4 + ttl
                    pv = ps.tile([128, QCH], FP32, tag="ps", bufs=4)
                    for ct in range(CT):
                        nc.tensor.matmul(
                            pv[:, :E], xt[:, ct, ttl * 128:(ttl + 1) * 128], wv_sb[:, ct, :],
                            start=(ct == 0), stop=(ct == CT - 1),
                        )
                    nc.vector.tensor_add(v_sb[:, tt, :], pv[:, :E], bv_sb)

            # ---- attention ----
            def st_exp(hl, kt, at, sums):
                """S^T tile (k in [128kt,128kt+128), ragged q) -> exp -> at."""
                hp, hrow = hl // 2, (hl % 2) * 64
                qc0, off, klo = kt // 4, 128 * (kt % 4), 128 * kt
                for qc in range(qc0, NQC):
                    lo = QCH * qc + (off if qc == qc0 else 0)
                    hi = QCH * qc + QCH
                    w = hi - lo
                    s_ps = ps.tile([128, QCH], FP32, tag="ps", bufs=4, name="s_ps")
                    nc.tensor.matmul(
                        s_ps[:, :w],
                        kt_sb[hrow:hrow + 64, hp, klo:klo + 128],
                        qt_sb[hrow:hrow + 64, hp, lo:hi],
                        start=True, stop=True,
                    )
                    if qc == qc0:
                        nc.vector.tensor_add(s_ps[:, :128], s_ps[:, :128], mask_sb)
                    nc.scalar.activation(
                        at[:, lo:hi], s_ps[:, :w], mybir.ActivationFunctionType.Exp,
                        scale=SCALE, accum_out=sums[:, qc:qc + 1],
                    )

            def norm_v(hl, kt, sums, qc0):
                stot = sm.tile([128, 1], FP32, tag="stot", bufs=4, name="stot")
                nc.vector.reduce_sum(stot, sums[:, qc0:NQC], axis=mybir.AxisListType.X)
                rcp = sm.tile([128, 1], FP32, tag="rcp", bufs=4, name="rcp")
                nc.vector.reciprocal(rcp, stot)
                vs = sm.tile([128, D], av_dt, tag="vs", bufs=4, name="vs")
                nc.vector.tensor_scalar_mul(vs, v_sb[:, kt, hl * 64:(hl + 1) * 64], rcp)
                return vs

            if not pair_bf16_av:
                for hl in range(HPC):
                    hp, hrow = hl // 2, (hl % 2) * 64
                    yps = psy.tile([64, T], FP32, tag="y", name="yps")
                    for kt in range(TT):
                        qc0, klo = kt // 4, 128 * kt
                        at = atp.tile([128, T], FP32R, tag="at", bufs=3, name="at")
                        sums = sm.tile([128, NQC], FP32, tag="sums", bufs=4, name="sums")
                        st_exp(hl, kt, at, sums)
                        vs = norm_v(hl, kt, sums, qc0)
                        for qc in range(qc0, NQC):
                            lo = max(QCH * qc, klo)
                            hi = QCH * qc + QCH
                            nc.tensor.matmul(
                                yps[:, lo:hi], vs, at[:, lo:hi],
                                start=(kt == 0), stop=(kt == min(TT - 1, 4 * qc + 3)),
                            )
                    nc.vector.tensor_copy(y_sb[hrow:hrow + 64, hp, :], yps)
            else:
                # paired heads: ST row-tiled (64x128), AV col-tiled bf16 (128x64)
                for hp in range(ET):
                    yps = psy.tile([128, T], FP32, tag="y", name="yps")
                    for kt in range(TT):
                        qc0, klo = kt // 4, 128 * kt
                        ats = []
                        for sub in range(2):
                            hl = 2 * hp + sub
                            at = atp.tile([128, T], BF16, tag="at", bufs=4, name="at")
                            sums = sm.tile([128, NQC], FP32, tag="sums", bufs=4, name="sums")
                            st_exp(hl, kt, at, sums)
                            vs = norm_v(hl, kt, sums, qc0)
                            ats.append((at, vs))
                        for qc in range(qc0, NQC):
                            lo = max(QCH * qc, klo)
                            hi = QCH * qc + QCH
                            for sub in range(2):
                                at, vs = ats[sub]
                                nc.tensor.matmul(
                                    yps[sub * 64:sub * 64 + 64, lo:hi], vs, at[:, lo:hi],
                                    start=(kt == 0), stop=(kt == min(TT - 1, 4 * qc + 3)),
                                )
                    nc.vector.tensor_copy(y_sb[:, hp, :], yps)

            # ---- output projection ----
            for tt in range(TT):
                po1 = ps.tile([128, QCH], FP32, tag="ps", bufs=4, name="po1")
                po2 = ps.tile([128, QCH], FP32, tag="ps", bufs=4, name="po2")
                for et in range(ET):
                    nc.tensor.matmul(
                        po1, y_sb[:, et, tt * 128:(tt + 1) * 128], wp_sb[:, et, 0:QCH],
                        start=(et == 0), stop=(et == ET - 1),
                    )
                    nc.tensor.matmul(
                        po2[:, :C - QCH], y_sb[:, et, tt * 128:(tt + 1) * 128],
                        wp_sb[:, et, QCH:C],
                        start=(et == 0), stop=(et == ET - 1),
                    )
                o_sb = op.tile([128, C], FP32, tag="o", bufs=3, name="o_sb")
                nc.vector.tensor_copy(o_sb[:, 0:QCH], po1)
                nc.vector.tensor_copy(o_sb[:, QCH:C], po2[:, :C - QCH])
                nc.sync.dma_start(out=out[tt * 128:(tt + 1) * 128, :], in_=o_sb)

    _split_sync_waits(nc)
    return nc


_nc_cache = {}
last_result = None


def kernel(x, w_attn, b_attn, w_proj, b_proj):
    global last_result
    pair = os.environ.get("ATT_PAIR_BF16", "0") == "1"
    if pair not in _nc_cache:
        _nc_cache[pair] = _build(pair)
    nc = _nc_cache[pair]

    x = np.asarray(x, dtype=np.float32)
    w_attn = np.asarray(w_attn, dtype=np.float32)
    b_attn = np.asarray(b_attn, dtype=np.float32)
    w_proj = np.asarray(w_proj, dtype=np.float32)
    b_proj = np.asarray(b_proj, dtype=np.float32)

    tri = np.where(
        np.arange(128)[None, :] >= np.arange(128)[:, None], 0.0, MASKV
    ).astype(np.float32)

    in_maps = []
    for core in range(NCORES):
        b = core // 2
        e0 = (core % 2) * E
        in_maps.append({
            "xT": np.ascontiguousarray(x[b].T),
            "wq": np.ascontiguousarray(w_attn[:, e0:e0 + E]),
            "wk": np.ascontiguousarray(w_attn[:, C + e0:C + e0 + E]),
            "wv": np.ascontiguousarray(w_attn[:, 2 * C + e0:2 * C + e0 + E]),
            "bq": np.ascontiguousarray(b_attn[e0:e0 + E]),
            "bk": np.ascontiguousarray(b_attn[C + e0:C + e0 + E]),
            "bv": np.ascontiguousarray(b_attn[2 * C + e0:2 * C + e0 + E]),
            "wp": np.ascontiguousarray(w_proj[e0:e0 + E, :]),
            "mask": tri,
        })

    trace = os.environ.get("ATT_TRACE", "0") == "1"
    kw = {}
    if trace:
        kw = dict(trace=True, trace_cores=list(range(NCORES)))
    res = run_bass_kernel_spmd(nc, in_maps, list(range(NCORES)), **kw)
    last_result = res

    out = np.zeros((B, T, C), dtype=np.float32)
    for core in range(NCORES):
        out[core // 2] += res.results[core]["out"]
    out += b_proj[None, None, :]
    return out


# revision 2
# speedup vs baseline: 1.0017x; 1.0017x over previous
"""Causal self-attention (query-axis softmax) for Trainium2, 8 NeuronCores.

Sharding: 8 cores = 4 batches x 2 half-head-groups. Core c handles batch
c//2 and heads (c%2)*6 .. (c%2)*6+5. Each core computes its heads' full
attention plus its partial output projection; the host sums the two
partials per batch and adds b_proj.

Layout strategy per core (T=2048, C=768, 6 heads, hd=64):
  - host passes x[b].T so the QKV contraction dim (C) lands on SBUF
    partitions without any on-chip transpose.
  - Q,K are produced transposed ([head_d, t]) so S^T = K Q^T tiles have
    softmax's query axis on the free dimension; V is produced in [t, d].
  - softmax over q (free axis): no max-subtraction needed (logits are
    O(1) by construction), exp+rowsum fused on ScalarE via accum_out,
    normalization folded into V rows (scale V[k,:] by 1/denom[k]).
  - causal mask: ragged chunk bounds skip fully-masked blocks; diagonal
    128x128 blocks get a precomputed triangular -30000 add.
  - all matmuls run as float32r (full PE rate at moving dim >= 256,
    ~1e-4 relative error).
"""

import os
import sys

sys.path.insert(0, "/opt/trn_rl_repo")

import numpy as np

import concourse.bass as bass
import concourse.mybir as mybir
import concourse.tile as tile
from concourse import bass_utils
from concourse.bass_utils import run_bass_kernel_spmd

# ST/AV/proj matmuls reuse the same stationary operand across consecutive
# instructions; walrus's redundant-LDWEIGHTS elision is off by default in
# this wrapper, and LDWEIGHTS otherwise costs ~200ns per matmul.
if not getattr(bass_utils, "_ldw_opt_patched", False):
    _orig_run_command = bass_utils.run_command

    def _run_command_ldw(cmd, *a, **kw):
        cmd = [
            "--enable-ldw-opt=true" if c == "--enable-ldw-opt=false" else c
            for c in cmd
        ]
        return _orig_run_command(cmd, *a, **kw)

    bass_utils.run_command = _run_command_ldw
    bass_utils._ldw_opt_patched = True

FP32 = mybir.dt.float32
FP32R = mybir.dt.float32r
BF16 = mybir.dt.bfloat16

B, T, C, H = 4, 2048, 768, 12
D = 64                  # head dim
NCORES = 8
HPC = H * B // NCORES   # heads per core = 6
E = HPC * D             # qkv slice width per core = 384
CT = C // 128           # c tiles = 6
ET = E // 128           # e tiles = 3
TT = T // 128           # t tiles = 16
QCH = 512               # q chunk
NQC = T // QCH          # q chunks = 4
MASKV = -30000.0
SCALE = 1.0 / 8.0       # 1/sqrt(hd)


def _split_sync_waits(nc):
    """This container's walrus encodes at most one sync wait per
    instruction for several instruction structs; hoist extra waits onto
    same-engine nops placed immediately before the instruction."""
    for f in nc.m.functions:
        for bb in f.blocks:
            new_insts = []
            for inst in bb.instructions:
                si = inst.sync_info
                waits = list(si.on_wait) if si is not None and si.on_wait else []
                if len(waits) > 1:
                    for w in waits[:-1]:
                        nop = mybir.InstNoOp(
                            name=nc.get_next_instruction_name(),
                            engine=inst.engine,
                            sync_info=mybir.SyncInfo(on_wait=[w], on_update=[]),
                            bass_nofuse=True,
                        )
                        nc.register_instruction(nop)
                        new_insts.append(nop)
                    inst.sync_info = mybir.SyncInfo(
                        on_wait=[waits[-1]], on_update=list(si.on_update or [])
                    )
                new_insts.append(inst)
            bb.instructions[:] = new_insts


def _build(pair_bf16_av: bool):
    nc = bass.Bass("TRN2")
    xT = nc.dram_tensor("xT", [C, T], FP32, kind="ExternalInput")
    wq = nc.dram_tensor("wq", [C, E], FP32, kind="ExternalInput")
    wk = nc.dram_tensor("wk", [C, E], FP32, kind="ExternalInput")
    wv = nc.dram_tensor("wv", [C, E], FP32, kind="ExternalInput")
    bq = nc.dram_tensor("bq", [E], FP32, kind="ExternalInput")
    bk = nc.dram_tensor("bk", [E], FP32, kind="ExternalInput")
    bv = nc.dram_tensor("bv", [E], FP32, kind="ExternalInput")
    wp = nc.dram_tensor("wp", [E, C], FP32, kind="ExternalInput")
    mask = nc.dram_tensor("mask", [128, 128], FP32, kind="ExternalInput")
    out = nc.dram_tensor("out", [T, C], FP32, kind="ExternalOutput")

    av_dt = BF16 if pair_bf16_av else FP32R

    with tile.TileContext(nc) as tc:
        with (
            tc.tile_pool(name="wts", bufs=1) as wts,
            tc.tile_pool(name="xp", bufs=2) as xp,
            tc.tile_pool(name="big", bufs=1) as big,
            tc.tile_pool(name="atp", bufs=3) as atp,
            tc.tile_pool(name="sm", bufs=4) as sm,
            tc.tile_pool(name="op", bufs=3) as op,
            tc.tile_pool(name="ps", bufs=4, space="PSUM") as ps,
            tc.tile_pool(name="psy", bufs=1, space="PSUM") as psy,
        ):
            # ---- constant loads ----
            wq_sb = wts.tile([128, CT, E], FP32R)
            wk_sb = wts.tile([128, CT, E], FP32R)
            wv_sb = wts.tile([128, CT, E], FP32R)
            nc.sync.dma_start(out=wq_sb, in_=wq.rearrange("(ct p) e -> p ct e", p=128).bitcast(FP32R))
            nc.sync.dma_start(out=wk_sb, in_=wk.rearrange("(ct p) e -> p ct e", p=128).bitcast(FP32R))
            nc.sync.dma_start(out=wv_sb, in_=wv.rearrange("(ct p) e -> p ct e", p=128).bitcast(FP32R))
            wp_sb = wts.tile([128, ET, C], FP32R)
            nc.sync.dma_start(out=wp_sb, in_=wp.rearrange("(et p) c -> p et c", p=128).bitcast(FP32R))
            bq_sb = wts.tile([128, ET], FP32)
            bk_sb = wts.tile([128, ET], FP32)
            nc.sync.dma_start(out=bq_sb, in_=bq.rearrange("(et p) -> p et", p=128))
            nc.sync.dma_start(out=bk_sb, in_=bk.rearrange("(et p) -> p et", p=128))
            bv_sb = wts.tile([128, E], FP32)
            nc.sync.dma_start(out=bv_sb, in_=bv[None, :].to_broadcast((128, E)))
            mask_sb = wts.tile([128, 128], FP32)
            nc.sync.dma_start(out=mask_sb, in_=mask[:])

            qt_sb = big.tile([128, ET, T], FP32R)   # [d-in-pair, pair, t]
            kt_sb = big.tile([128, ET, T], FP32R)
            v_sb = big.tile([128, TT, E], av_dt)    # [t-in-tile, ttile, (head,d)]
            y_sb = big.tile([128, ET, T], FP32R)    # [hd-in-pair, pair, t]

            # ---- QKV ----
            for tci in range(NQC):
                xt = xp.tile([128, CT, QCH], FP32R, tag="xt", bufs=2)
                nc.sync.dma_start(
                    out=xt,
                    in_=xT[:, tci * QCH:(tci + 1) * QCH]
                    .rearrange("(ct p) t -> p ct t", p=128).bitcast(FP32R),
                )
                for et in range(ET):
                    pq = ps.tile([128, QCH], FP32, tag="ps", bufs=4)
                    for ct in range(CT):
                        nc.tensor.matmul(
                            pq, wq_sb[:, ct, et * 128:(et + 1) * 128], xt[:, ct, :],
                            start=(ct == 0), stop=(ct == CT - 1),
                        )
                    nc.vector.tensor_scalar_add(
                        qt_sb[:, et, tci * QCH:(tci + 1) * QCH], pq, bq_sb[:, et:et + 1]
                    )
                    pk = ps.tile([128, QCH], FP32, tag="ps", bufs=4)
                    for ct in range(CT):
                        nc.tensor.matmul(
                            pk, wk_sb[:, ct, et * 128:(et + 1) * 128], xt[:, ct, :],
                            start=(ct == 0), stop=(ct == CT - 1),
                        )
                    nc.vector.tensor_scalar_add(
                        kt_sb[:, et, tci * QCH:(tci + 1) * QCH], pk, bk_sb[:, et:et + 1]
                    )
                for ttl in range(4):
                    tt = tci * 4 + ttl
                    pv = ps.tile([128, QCH], FP32, tag="ps", bufs=4)
                    for ct in range(CT):
                        nc.tensor.matmul(
                            pv[:, :E], xt[:, ct, ttl * 128:(ttl + 1) * 128], wv_sb[:, ct, :],
                            start=(ct == 0), stop=(ct == CT - 1),
                        )
                    nc.vector.tensor_add(v_sb[:, tt, :], pv[:, :E], bv_sb)

            # ---- attention ----
            def st_exp(hl, kt, at, sums):
                """S^T tile (k in [128kt,128kt+128), ragged q) -> exp -> at."""
                hp, hrow = hl // 2, (hl % 2) * 64
                qc0, off, klo = kt // 4, 128 * (kt % 4), 128 * kt
                for qc in range(qc0, NQC):
                    lo = QCH * qc + (off if qc == qc0 else 0)
                    hi = QCH * qc + QCH
                    w = hi - lo
                    s_ps = ps.tile([128, QCH], FP32, tag="ps", bufs=4, name="s_ps")
                    nc.tensor.matmul(
                        s_ps[:, :w],
                        kt_sb[hrow:hrow + 64, hp, klo:klo + 128],
                        qt_sb[hrow:hrow + 64, hp, lo:hi],
                        start=True, stop=True,
                    )
                    if qc == qc0:
                        nc.vector.tensor_add(s_ps[:, :128], s_ps[:, :128], mask_sb)
                    nc.scalar.activation(
                        at[:, lo:hi], s_ps[:, :w], mybir.ActivationFunctionType.Exp,
                        scale=SCALE, accum_out=sums[:, qc:qc + 1],
                    )

            def norm_v(hl, kt, sums, qc0):
                stot = sm.tile([128, 1], FP32, tag="stot", bufs=4, name="stot")
                nc.vector.reduce_sum(stot, sums[:, qc0:NQC], axis=mybir.AxisListType.X)
                rcp = sm.tile([128, 1], FP32, tag="rcp", bufs=4, name="rcp")
                nc.vector.reciprocal(rcp, stot)
                vs = sm.tile([128, D], av_dt, tag="vs", bufs=4, name="vs")
                nc.vector.tensor_scalar_mul(vs, v_sb[:, kt, hl * 64:(hl + 1) * 64], rcp)
                return vs

            if not pair_bf16_av:
                for hl in range(HPC):
                    hp, hrow = hl // 2, (hl % 2) * 64
                    yps = psy.tile([64, T], FP32, tag="y", name="yps")
                    for kt in range(TT):
                        qc0, klo = kt // 4, 128 * kt
                        at = atp.tile([128, T], FP32R, tag="at", bufs=3, name="at")
                        sums = sm.tile([128, NQC], FP32, tag="sums", bufs=4, name="sums")
                        st_exp(hl, kt, at, sums)
                        vs = norm_v(hl, kt, sums, qc0)
                        for qc in range(qc0, NQC):
                            lo = max(QCH * qc, klo)
                            hi = QCH * qc + QCH
                            nc.tensor.matmul(
                                yps[:, lo:hi], vs, at[:, lo:hi],
                                start=(kt == 0), stop=(kt == min(TT - 1, 4 * qc + 3)),
                            )
                    nc.vector.tensor_copy(y_sb[hrow:hrow + 64, hp, :], yps)
            else:
                # paired heads: ST row-tiled (64x128), AV col-tiled bf16 (128x64)
                for hp in range(ET):
                    yps = psy.tile([128, T], FP32, tag="y", name="yps")
                    for kt in range(TT):
                        qc0, klo = kt // 4, 128 * kt
                        ats = []
                        for sub in range(2):
                            hl = 2 * hp + sub
                            at = atp.tile([128, T], BF16, tag="at", bufs=4, name="at")
                            sums = sm.tile([128, NQC], FP32, tag="sums", bufs=4, name="sums")
                            st_exp(hl, kt, at, sums)
                            vs = norm_v(hl, kt, sums, qc0)
                            ats.append((at, vs))
                        for qc in range(qc0, NQC):
                            lo = max(QCH * qc, klo)
                            hi = QCH * qc + QCH
                            for sub in range(2):
                                at, vs = ats[sub]
                                nc.tensor.matmul(
                                    yps[sub * 64:sub * 64 + 64, lo:hi], vs, at[:, lo:hi],
                                    start=(kt == 0), stop=(kt == min(TT - 1, 4 * qc + 3)),
                                )
                    nc.vector.tensor_copy(y_sb[:, hp, :], yps)

            # ---- output projection ----
            for tt in range(TT):
                po1 = ps.tile([128, QCH], FP32, tag="ps", bufs=4, name="po1")
                po2 = ps.tile([128, QCH], FP32, tag="ps", bufs=4, name="po2")
                for et in range(ET):
                    nc.tensor.matmul(
                        po1, y_sb[:, et, tt * 128:(tt + 1) * 128], wp_sb[:, et, 0:QCH],
                        start=(et == 0), stop=(et == ET - 1),
                    )
                    nc.tensor.matmul(
                        po2[:, :C - QCH], y_sb[:, et, tt * 128:(tt + 1) * 128],
                        wp_sb[:, et, QCH:C],
                        start=(et == 0), stop=(et == ET - 1),
                    )
                o_sb = op.tile([128, C], FP32, tag="o", bufs=3, name="o_sb")
                nc.vector.tensor_copy(o_sb[:, 0:QCH], po1)
                nc.vector.tensor_copy(o_sb[:, QCH:C], po2[:, :C - QCH])
                nc.sync.dma_start(out=out[tt * 128:(tt + 1) * 128, :], in_=o_sb)

    _split_sync_waits(nc)
    return nc


_nc_cache = {}
last_result = None


def kernel(x, w_attn, b_attn, w_proj, b_proj):
    global last_result
    pair = os.environ.get("ATT_PAIR_BF16", "0") == "1"
    if pair not in _nc_cache:
        _nc_cache[pair] = _build(pair)
    nc = _nc_cache[pair]

    x = np.asarray(x, dtype=np.float32)
    w_attn = np.asarray(w_attn, dtype=np.float32)
    b_attn = np.asarray(b_attn, dtype=np.float32)
    w_proj = np.asarray(w_proj, dtype=np.float32)
    b_proj = np.asarray(b_proj, dtype=np.float32)

    tri = np.where(
        np.arange(128)[None, :] >= np.arange(128)[:, None], 0.0, MASKV
    ).astype(np.float32)

    in_maps = []
    for core in range(NCORES):
        b = core // 2
        e0 = (core % 2) * E
        in_maps.append({
            "xT": np.ascontiguousarray(x[b].T),
            "wq": np.ascontiguousarray(w_attn[:, e0:e0 + E]),
            "wk": np.ascontiguousarray(w_attn[:, C + e0:C + e0 + E]),
            "wv": np.ascontiguousarray(w_attn[:, 2 * C + e0:2 * C + e0 + E]),
            "bq": np.ascontiguousarray(b_attn[e0:e0 + E]),
            "bk": np.ascontiguousarray(b_attn[C + e0:C + e0 + E]),
            "bv": np.ascontiguousarray(b_attn[2 * C + e0:2 * C + e0 + E]),
            "wp": np.ascontiguousarray(w_proj[e0:e0 + E, :]),
            "mask": tri,
        })

    trace = os.environ.get("ATT_TRACE", "0") == "1"
    kw = {}
    if trace:
        kw = dict(trace=True, trace_cores=list(range(NCORES)))
    res = run_bass_kernel_spmd(nc, in_maps, list(range(NCORES)), **kw)
    last_result = res

    out = np.zeros((B, T, C), dtype=np.float32)
    for core in range(NCORES):
        out[core // 2] += res.results[core]["out"]
    out += b_proj[None, None, :]
    return out


# revision 3
# speedup vs baseline: 1.1170x; 1.1152x over previous
"""Causal self-attention (query-axis softmax) for Trainium2, 8 NeuronCores.

Sharding: 8 cores = 4 batches x 2 half-head-groups. Core c handles batch
c//2 and heads (c%2)*6 .. (c%2)*6+5. Each core computes its heads' full
attention plus its partial output projection; the host sums the two
partials per batch and adds b_proj.

Layout strategy per core (T=2048, C=768, 6 heads, hd=64):
  - host passes x[b].T so the QKV contraction dim (C) lands on SBUF
    partitions without any on-chip transpose.
  - Q,K are produced transposed ([head_d, t]) so S^T = K Q^T tiles have
    softmax's query axis on the free dimension; V is produced in [t, d].
  - softmax over q (free axis): no max-subtraction needed (logits are
    O(1) by construction), exp+rowsum fused on ScalarE via accum_out,
    normalization folded into V rows (scale V[k,:] by 1/denom[k]).
  - causal mask: ragged chunk bounds skip fully-masked blocks; diagonal
    128x128 blocks get a precomputed triangular -30000 add.
  - all matmuls run as float32r (full PE rate at moving dim >= 256,
    ~1e-4 relative error).
"""

import os
import sys

sys.path.insert(0, "/opt/trn_rl_repo")

import numpy as np

import concourse.bass as bass
import concourse.mybir as mybir
import concourse.tile as tile
from concourse import bass_utils
from concourse.bass_utils import run_bass_kernel_spmd

# ST/AV/proj matmuls reuse the same stationary operand across consecutive
# instructions; walrus's redundant-LDWEIGHTS elision is off by default in
# this wrapper, and LDWEIGHTS otherwise costs ~200ns per matmul.
LDW_OPT = {"on": True}
if not getattr(bass_utils, "_ldw_opt_patched", False):
    _orig_run_command = bass_utils.run_command

    def _run_command_ldw(cmd, *a, **kw):
        if LDW_OPT["on"]:
            cmd = [
                "--enable-ldw-opt=true" if c == "--enable-ldw-opt=false" else c
                for c in cmd
            ]
        return _orig_run_command(cmd, *a, **kw)

    bass_utils.run_command = _run_command_ldw
    bass_utils._ldw_opt_patched = True

FP32 = mybir.dt.float32
FP32R = mybir.dt.float32r
BF16 = mybir.dt.bfloat16

B, T, C, H = 4, 2048, 768, 12
D = 64                  # head dim
NCORES = 8
HPC = H * B // NCORES   # heads per core = 6
E = HPC * D             # qkv slice width per core = 384
CT = C // 128           # c tiles = 6
ET = E // 128           # e tiles = 3
TT = T // 128           # t tiles = 16
QCH = 512               # q chunk
NQC = T // QCH          # q chunks = 4
MASKV = -30000.0
SCALE = 1.0 / 8.0       # 1/sqrt(hd)


def _split_sync_waits(nc):
    """This container's walrus encodes at most one sync wait per
    instruction for several instruction structs; hoist extra waits onto
    same-engine nops placed immediately before the instruction."""
    for f in nc.m.functions:
        for bb in f.blocks:
            new_insts = []
            for inst in bb.instructions:
                si = inst.sync_info
                waits = list(si.on_wait) if si is not None and si.on_wait else []
                if len(waits) > 1:
                    for w in waits[:-1]:
                        nop = mybir.InstNoOp(
                            name=nc.get_next_instruction_name(),
                            engine=inst.engine,
                            sync_info=mybir.SyncInfo(on_wait=[w], on_update=[]),
                            bass_nofuse=True,
                        )
                        nc.register_instruction(nop)
                        new_insts.append(nop)
                    inst.sync_info = mybir.SyncInfo(
                        on_wait=[waits[-1]], on_update=list(si.on_update or [])
                    )
                new_insts.append(inst)
            bb.instructions[:] = new_insts


def _build(pair_bf16_av: bool):
    nc = bass.Bass("TRN2")
    xT = nc.dram_tensor("xT", [C, T], FP32, kind="ExternalInput")
    wq = nc.dram_tensor("wq", [C, E], FP32, kind="ExternalInput")
    wk = nc.dram_tensor("wk", [C, E], FP32, kind="ExternalInput")
    wv = nc.dram_tensor("wv", [C, E], FP32, kind="ExternalInput")
    bq = nc.dram_tensor("bq", [E], FP32, kind="ExternalInput")
    bk = nc.dram_tensor("bk", [E], FP32, kind="ExternalInput")
    bv = nc.dram_tensor("bv", [E], FP32, kind="ExternalInput")
    wp = nc.dram_tensor("wp", [E, C], FP32, kind="ExternalInput")
    mask = nc.dram_tensor("mask", [128, 128], FP32, kind="ExternalInput")
    out = nc.dram_tensor("out", [T, C], FP32, kind="ExternalOutput")

    av_dt = BF16 if pair_bf16_av else FP32R

    with tile.TileContext(nc) as tc:
        with (
            tc.tile_pool(name="wts", bufs=1) as wts,
            tc.tile_pool(name="xp", bufs=2) as xp,
            tc.tile_pool(name="big", bufs=1) as big,
            tc.tile_pool(name="atp", bufs=3) as atp,
            tc.tile_pool(name="sm", bufs=4) as sm,
            tc.tile_pool(name="op", bufs=3) as op,
            tc.tile_pool(name="ps", bufs=4, space="PSUM") as ps,
            tc.tile_pool(name="psy", bufs=1, space="PSUM") as psy,
        ):
            # ---- constant loads ----
            wq_sb = wts.tile([128, CT, E], FP32R)
            wk_sb = wts.tile([128, CT, E], FP32R)
            wv_sb = wts.tile([128, CT, E], FP32R)
            nc.sync.dma_start(out=wq_sb, in_=wq.rearrange("(ct p) e -> p ct e", p=128).bitcast(FP32R))
            nc.sync.dma_start(out=wk_sb, in_=wk.rearrange("(ct p) e -> p ct e", p=128).bitcast(FP32R))
            nc.sync.dma_start(out=wv_sb, in_=wv.rearrange("(ct p) e -> p ct e", p=128).bitcast(FP32R))
            wp_sb = wts.tile([128, ET, C], FP32R)
            nc.sync.dma_start(out=wp_sb, in_=wp.rearrange("(et p) c -> p et c", p=128).bitcast(FP32R))
            bq_sb = wts.tile([128, ET], FP32)
            bk_sb = wts.tile([128, ET], FP32)
            nc.sync.dma_start(out=bq_sb, in_=bq.rearrange("(et p) -> p et", p=128))
            nc.sync.dma_start(out=bk_sb, in_=bk.rearrange("(et p) -> p et", p=128))
            bv_sb = wts.tile([128, E], FP32)
            nc.sync.dma_start(out=bv_sb, in_=bv[None, :].to_broadcast((128, E)))
            mask_sb = wts.tile([128, 128], FP32)
            nc.sync.dma_start(out=mask_sb, in_=mask[:])

            qt_sb = big.tile([128, ET, T], FP32R)   # [d-in-pair, pair, t]
            kt_sb = big.tile([128, ET, T], FP32R)
            v_sb = big.tile([128, TT, E], av_dt)    # [t-in-tile, ttile, (head,d)]
            y_sb = big.tile([128, ET, T], FP32R)    # [hd-in-pair, pair, t]

            # ---- QKV ----
            for tci in range(NQC):
                xt = xp.tile([128, CT, QCH], FP32R, tag="xt", bufs=2)
                nc.sync.dma_start(
                    out=xt,
                    in_=xT[:, tci * QCH:(tci + 1) * QCH]
                    .rearrange("(ct p) t -> p ct t", p=128).bitcast(FP32R),
                )
                for et in range(ET):
                    pq = ps.tile([128, QCH], FP32, tag="ps", bufs=4)
                    for ct in range(CT):
                        nc.tensor.matmul(
                            pq, wq_sb[:, ct, et * 128:(et + 1) * 128], xt[:, ct, :],
                            start=(ct == 0), stop=(ct == CT - 1),
                        )
                    nc.vector.tensor_scalar_add(
                        qt_sb[:, et, tci * QCH:(tci + 1) * QCH], pq, bq_sb[:, et:et + 1]
                    )
                    pk = ps.tile([128, QCH], FP32, tag="ps", bufs=4)
                    for ct in range(CT):
                        nc.tensor.matmul(
                            pk, wk_sb[:, ct, et * 128:(et + 1) * 128], xt[:, ct, :],
                            start=(ct == 0), stop=(ct == CT - 1),
                        )
                    nc.vector.tensor_scalar_add(
                        kt_sb[:, et, tci * QCH:(tci + 1) * QCH], pk, bk_sb[:, et:et + 1]
                    )
                for ttl in range(4):
                    tt = tci * 4 + ttl
                    pv = ps.tile([128, QCH], FP32, tag="ps", bufs=4)
                    for ct in range(CT):
                        nc.tensor.matmul(
                            pv[:, :E], xt[:, ct, ttl * 128:(ttl + 1) * 128], wv_sb[:, ct, :],
                            start=(ct == 0), stop=(ct == CT - 1),
                        )
                    nc.vector.tensor_add(v_sb[:, tt, :], pv[:, :E], bv_sb)

            # ---- attention ----
            def st_exp(hl, kt, at, sums):
                """S^T tile (k in [128kt,128kt+128), ragged q) -> exp -> at."""
                hp, hrow = hl // 2, (hl % 2) * 64
                qc0, off, klo = kt // 4, 128 * (kt % 4), 128 * kt
                for qc in range(qc0, NQC):
                    lo = QCH * qc + (off if qc == qc0 else 0)
                    hi = QCH * qc + QCH
                    w = hi - lo
                    s_ps = ps.tile([128, QCH], FP32, tag="ps", bufs=4, name="s_ps")
                    nc.tensor.matmul(
                        s_ps[:, :w],
                        kt_sb[hrow:hrow + 64, hp, klo:klo + 128],
                        qt_sb[hrow:hrow + 64, hp, lo:hi],
                        start=True, stop=True,
                    )
                    if qc == qc0:
                        nc.vector.tensor_add(s_ps[:, :128], s_ps[:, :128], mask_sb)
                    nc.scalar.activation(
                        at[:, lo:hi], s_ps[:, :w], mybir.ActivationFunctionType.Exp,
                        scale=SCALE, accum_out=sums[:, qc:qc + 1],
                    )

            def norm_v(hl, kt, sums, qc0):
                stot = sm.tile([128, 1], FP32, tag="stot", bufs=4, name="stot")
                nc.vector.reduce_sum(stot, sums[:, qc0:NQC], axis=mybir.AxisListType.X)
                rcp = sm.tile([128, 1], FP32, tag="rcp", bufs=4, name="rcp")
                nc.vector.reciprocal(rcp, stot)
                vs = sm.tile([128, D], av_dt, tag="vs", bufs=4, name="vs")
                nc.vector.tensor_scalar_mul(vs, v_sb[:, kt, hl * 64:(hl + 1) * 64], rcp)
                return vs

            if not pair_bf16_av:
                for hl in range(HPC):
                    hp, hrow = hl // 2, (hl % 2) * 64
                    yps = psy.tile([64, T], FP32, tag="y", name="yps")
                    for kt in range(TT):
                        qc0, klo = kt // 4, 128 * kt
                        at = atp.tile([128, T], FP32R, tag="at", bufs=3, name="at")
                        sums = sm.tile([128, NQC], FP32, tag="sums", bufs=4, name="sums")
                        st_exp(hl, kt, at, sums)
                        vs = norm_v(hl, kt, sums, qc0)
                        for qc in range(qc0, NQC):
                            lo = max(QCH * qc, klo)
                            hi = QCH * qc + QCH
                            nc.tensor.matmul(
                                yps[:, lo:hi], vs, at[:, lo:hi],
                                start=(kt == 0), stop=(kt == min(TT - 1, 4 * qc + 3)),
                            )
                    nc.vector.tensor_copy(y_sb[hrow:hrow + 64, hp, :], yps)
            else:
                # paired heads: ST row-tiled (64x128), AV col-tiled bf16 (128x64)
                for hp in range(ET):
                    yps = psy.tile([128, T], FP32, tag="y", name="yps")
                    for kt in range(TT):
                        qc0, klo = kt // 4, 128 * kt
                        ats = []
                        for sub in range(2):
                            hl = 2 * hp + sub
                            at = atp.tile([128, T], BF16, tag="at", bufs=4, name="at")
                            sums = sm.tile([128, NQC], FP32, tag="sums", bufs=4, name="sums")
                            st_exp(hl, kt, at, sums)
                            vs = norm_v(hl, kt, sums, qc0)
                            ats.append((at, vs))
                        for qc in range(qc0, NQC):
                            lo = max(QCH * qc, klo)
                            hi = QCH * qc + QCH
                            for sub in range(2):
                                at, vs = ats[sub]
                                nc.tensor.matmul(
                                    yps[sub * 64:sub * 64 + 64, lo:hi], vs, at[:, lo:hi],
                                    start=(kt == 0), stop=(kt == min(TT - 1, 4 * qc + 3)),
                                )
                    nc.vector.tensor_copy(y_sb[:, hp, :], yps)

            # ---- output projection ----
            for tt in range(TT):
                po1 = ps.tile([128, QCH], FP32, tag="ps", bufs=4, name="po1")
                po2 = ps.tile([128, QCH], FP32, tag="ps", bufs=4, name="po2")
                for et in range(ET):
                    nc.tensor.matmul(
                        po1, y_sb[:, et, tt * 128:(tt + 1) * 128], wp_sb[:, et, 0:QCH],
                        start=(et == 0), stop=(et == ET - 1),
                    )
                    nc.tensor.matmul(
                        po2[:, :C - QCH], y_sb[:, et, tt * 128:(tt + 1) * 128],
                        wp_sb[:, et, QCH:C],
                        start=(et == 0), stop=(et == ET - 1),
                    )
                o_sb = op.tile([128, C], FP32, tag="o", bufs=3, name="o_sb")
                nc.vector.tensor_copy(o_sb[:, 0:QCH], po1)
                nc.vector.tensor_copy(o_sb[:, QCH:C], po2[:, :C - QCH])
                nc.sync.dma_start(out=out[tt * 128:(tt + 1) * 128, :], in_=o_sb)

    _split_sync_waits(nc)
    return nc


_nc_cache = {}
last_result = None


def kernel(x, w_attn, b_attn, w_proj, b_proj):
    global last_result
    pair = os.environ.get("ATT_PAIR_BF16", "0") == "1"
    LDW_OPT["on"] = not pair
    if pair not in _nc_cache:
        _nc_cache[pair] = _build(pair)
    nc = _nc_cache[pair]

    x = np.asarray(x, dtype=np.float32)
    w_attn = np.asarray(w_attn, dtype=np.float32)
    b_attn = np.asarray(b_attn, dtype=np.float32)
    w_proj = np.asarray(w_proj, dtype=np.float32)
    b_proj = np.asarray(b_proj, dtype=np.float32)

    tri = np.where(
        np.arange(128)[None, :] >= np.arange(128)[:, None], 0.0, MASKV
    ).astype(np.float32)

    in_maps = []
    for core in range(NCORES):
        b = core // 2
        e0 = (core % 2) * E
        in_maps.append({
            "xT": np.ascontiguousarray(x[b].T),
            "wq": np.ascontiguousarray(w_attn[:, e0:e0 + E]),
            "wk": np.ascontiguousarray(w_attn[:, C + e0:C + e0 + E]),
            "wv": np.ascontiguousarray(w_attn[:, 2 * C + e0:2 * C + e0 + E]),
            "bq": np.ascontiguousarray(b_attn[e0:e0 + E]),
            "bk": np.ascontiguousarray(b_attn[C + e0:C + e0 + E]),
            "bv": np.ascontiguousarray(b_attn[2 * C + e0:2 * C + e0 + E]),
            "wp": np.ascontiguousarray(w_proj[e0:e0 + E, :]),
            "mask": tri,
        })

    trace = os.environ.get("ATT_TRACE", "0") == "1"
    kw = {}
    if trace:
        kw = dict(trace=True, trace_cores=list(range(NCORES)))
    res = run_bass_kernel_spmd(nc, in_maps, list(range(NCORES)), **kw)
    last_result = res

    out = np.zeros((B, T, C), dtype=np.float32)
    for core in range(NCORES):
        out[core // 2] += res.results[core]["out"]
    out += b_proj[None, None, :]
    return out


# revision 6
# speedup vs baseline: 1.2963x; 1.1605x over previous
"""Causal self-attention (query-axis softmax) for Trainium2, 8 NeuronCores.

Sharding: 8 cores = 4 batches x 2 half-head-groups. Core c handles batch
c//2 and heads (c%2)*6 .. (c%2)*6+5. Each core computes its heads' full
attention plus its partial output projection; the host sums the two
partials per batch and adds b_proj.

Layout strategy per core (T=2048, C=768, 6 heads, hd=64):
  - host passes x[b].T so the QKV contraction dim (C) lands on SBUF
    partitions without any on-chip transpose.
  - Q,K are produced transposed ([head_d, t]) so S^T = K Q^T tiles have
    softmax's query axis on the free dimension; V is produced in [t, d].
  - head_dim=64 would leave the PE half idle (and tiled 64-row matmul
    modes run at the cold 1.2 GHz clock - they do not feed the HAM
    activity monitor), so the 64-wide operands are zero-padded to full
    128x128 mode: K^T tiles carry zeros on the other head's partition
    rows, Vs tiles carry 64 zero columns. Full-mode matmuls stream at
    1 cycle/row and keep the clock at 2.4 GHz.
  - softmax over q (free axis): no max-subtraction needed (logits are
    O(1) by construction), exp+rowsum fused on ScalarE via accum_out
    over 1024-wide PSUM chunks, normalization folded into V rows
    (scale V[k,:] by 1/denom[k]).
  - causal mask: ragged chunk bounds skip fully-masked blocks; diagonal
    128x128 blocks get a precomputed triangular -30000 add.
  - S^T matmuls run in bf16 (softmax normalization cancels the logit
    rounding; ~0.1% effect), everything else float32r (~1e-4).
"""

import os
import sys

sys.path.insert(0, "/opt/trn_rl_repo")

import numpy as np

import concourse.bass as bass
import concourse.mybir as mybir
import concourse.tile as tile
from concourse import bass_utils
from concourse.bass_utils import run_bass_kernel_spmd

# Consecutive matmuls often reuse the same stationary operand; walrus's
# redundant-LDWEIGHTS elision is off by default in this wrapper and
# LDWEIGHTS otherwise costs ~200ns per matmul.
LDW_OPT = {"on": False}  # bf16 FWL LDWEIGHTS is incompatible with walrus ldw-opt
if not getattr(bass_utils, "_ldw_opt_patched", False):
    _orig_run_command = bass_utils.run_command

    def _run_command_ldw(cmd, *a, **kw):
        if LDW_OPT["on"]:
            cmd = [
                "--enable-ldw-opt=true" if c == "--enable-ldw-opt=false" else c
                for c in cmd
            ]
        return _orig_run_command(cmd, *a, **kw)

    bass_utils.run_command = _run_command_ldw
    bass_utils._ldw_opt_patched = True

FP32 = mybir.dt.float32
FP32R = mybir.dt.float32r
BF16 = mybir.dt.bfloat16

B, T, C, H = 4, 2048, 768, 12
D = 64                  # head dim
NCORES = 8
HPC = H * B // NCORES   # heads per core = 6
E = HPC * D             # qkv slice width per core = 384
CT = C // 128           # c tiles = 6
ET = E // 128           # e tiles = 3
TT = T // 128           # t tiles = 16
QCH = 512               # matmul moving chunk (PSUM bank limit)
NQC = T // QCH          # 4
BCH = 1024              # exp chunk
NBC = T // BCH          # 2
MASKV = -30000.0
SCALE = 1.0 / 8.0       # 1/sqrt(hd)
Exp = mybir.ActivationFunctionType.Exp


def _split_sync_waits(nc):
    """This container's walrus encodes at most one sync wait per
    instruction for several instruction structs; hoist extra waits onto
    same-engine nops placed immediately before the instruction."""
    for f in nc.m.functions:
        for bb in f.blocks:
            new_insts = []
            for inst in bb.instructions:
                si = inst.sync_info
                waits = list(si.on_wait) if si is not None and si.on_wait else []
                if len(waits) > 1:
                    for w in waits[:-1]:
                        nop = mybir.InstNoOp(
                            name=nc.get_next_instruction_name(),
                            engine=inst.engine,
                            sync_info=mybir.SyncInfo(on_wait=[w], on_update=[]),
                            bass_nofuse=True,
                        )
                        nc.register_instruction(nop)
                        new_insts.append(nop)
                    inst.sync_info = mybir.SyncInfo(
                        on_wait=[waits[-1]], on_update=list(si.on_update or [])
                    )
                new_insts.append(inst)
            bb.instructions[:] = new_insts


def _build():
    nc = bass.Bass("TRN2")
    xT = nc.dram_tensor("xT", [C, T], FP32, kind="ExternalInput")
    wq = nc.dram_tensor("wq", [C, E], FP32, kind="ExternalInput")
    wk = nc.dram_tensor("wk", [C, E], FP32, kind="ExternalInput")
    wv = nc.dram_tensor("wv", [C, E], FP32, kind="ExternalInput")
    bq = nc.dram_tensor("bq", [E], FP32, kind="ExternalInput")
    bk = nc.dram_tensor("bk", [E], FP32, kind="ExternalInput")
    bv = nc.dram_tensor("bv", [E], FP32, kind="ExternalInput")
    wp = nc.dram_tensor("wp", [E, C], FP32, kind="ExternalInput")
    mask = nc.dram_tensor("mask", [128, 128], FP32, kind="ExternalInput")
    out = nc.dram_tensor("out", [T, C], FP32, kind="ExternalOutput")

    with tile.TileContext(nc) as tc:
        with (
            tc.tile_pool(name="wts", bufs=1) as wts,
            tc.tile_pool(name="xp", bufs=2) as xp,
            tc.tile_pool(name="big", bufs=1) as big,
            tc.tile_pool(name="atp", bufs=3) as atp,
            tc.tile_pool(name="sm", bufs=4) as sm,
            tc.tile_pool(name="op", bufs=3) as op,
        ):
            # ---- constant loads ----
            wq_sb = wts.tile([128, CT, E], FP32R)
            wk_sb = wts.tile([128, CT, E], FP32R)
            wv_sb = wts.tile([128, CT, E], FP32R)
            nc.sync.dma_start(out=wq_sb, in_=wq.rearrange("(ct p) e -> p ct e", p=128).bitcast(FP32R))
            nc.sync.dma_start(out=wk_sb, in_=wk.rearrange("(ct p) e -> p ct e", p=128).bitcast(FP32R))
            nc.sync.dma_start(out=wv_sb, in_=wv.rearrange("(ct p) e -> p ct e", p=128).bitcast(FP32R))
            wp_sb = wts.tile([128, ET, C], FP32R)
            nc.sync.dma_start(out=wp_sb, in_=wp.rearrange("(et p) c -> p et c", p=128).bitcast(FP32R))
            bq_sb = wts.tile([128, ET], FP32)
            bk_sb = wts.tile([128, ET], FP32)
            nc.sync.dma_start(out=bq_sb, in_=bq.rearrange("(et p) -> p et", p=128))
            nc.sync.dma_start(out=bk_sb, in_=bk.rearrange("(et p) -> p et", p=128))
            bv_sb = wts.tile([128, E], FP32)
            nc.sync.dma_start(out=bv_sb, in_=bv[None, :].to_broadcast((128, E)))
            mask_sb = wts.tile([128, 128], FP32)
            nc.sync.dma_start(out=mask_sb, in_=mask[:])

            qt_sb = big.tile([128, ET, T], BF16)    # [d-in-pair, pair, t]
            kt_pad = big.tile([128, HPC, T], BF16)  # [d(+zero parity half), head, t]
            v_sb = big.tile([128, TT, E], FP32R)    # [t-in-tile, ttile, (head,d)]
            y_sb = big.tile([128, ET, T], FP32R)    # [hd-in-pair, pair, t]
            # zero the off-parity halves of kt_pad once; evacuations only
            # ever write the data halves.
            nc.gpsimd.memset(kt_pad.bitcast(FP32), 0.0)
            # persistent double-buffered Vs tiles; cols 64:128 stay zero.
            vspad = [big.tile([128, 128], FP32R, name=f"vspad{i}") for i in range(2)]
            for t_ in vspad:
                nc.gpsimd.memset(t_.bitcast(FP32), 0.0)

            # ---- QKV ----
            with tc.tile_pool(name="psA", bufs=4, space="PSUM") as psA:
                for tci in range(NQC):
                    xt = xp.tile([128, CT, QCH], FP32R, tag="xt", bufs=2)
                    nc.sync.dma_start(
                        out=xt,
                        in_=xT[:, tci * QCH:(tci + 1) * QCH]
                        .rearrange("(ct p) t -> p ct t", p=128).bitcast(FP32R),
                    )
                    cols = slice(tci * QCH, (tci + 1) * QCH)
                    for et in range(ET):
                        pq = psA.tile([128, QCH], FP32, tag="ps", bufs=4, name="pq")
                        for ct in range(CT):
                            nc.tensor.matmul(
                                pq, wq_sb[:, ct, et * 128:(et + 1) * 128], xt[:, ct, :],
                                start=(ct == 0), stop=(ct == CT - 1),
                            )
                        nc.vector.tensor_scalar_add(qt_sb[:, et, cols], pq, bq_sb[:, et:et + 1])
                        pk = psA.tile([128, QCH], FP32, tag="ps", bufs=4, name="pk")
                        for ct in range(CT):
                            nc.tensor.matmul(
                                pk, wk_sb[:, ct, et * 128:(et + 1) * 128], xt[:, ct, :],
                                start=(ct == 0), stop=(ct == CT - 1),
                            )
                        # split the e-row pair into per-head zero-padded slots
                        nc.vector.tensor_scalar_add(
                            kt_pad[0:64, 2 * et, cols], pk[0:64, :], bk_sb[0:64, et:et + 1]
                        )
                        nc.vector.tensor_scalar_add(
                            kt_pad[64:128, 2 * et + 1, cols], pk[64:128, :], bk_sb[64:128, et:et + 1]
                        )
                    for ttl in range(4):
                        tt = tci * 4 + ttl
                        pv = psA.tile([128, QCH], FP32, tag="ps", bufs=4, name="pv")
                        for ct in range(CT):
                            nc.tensor.matmul(
                                pv[:, :E], xt[:, ct, ttl * 128:(ttl + 1) * 128], wv_sb[:, ct, :],
                                start=(ct == 0), stop=(ct == CT - 1),
                            )
                        nc.vector.tensor_add(v_sb[:, tt, :], pv[:, :E], bv_sb)

            # ---- attention ----
            with (
                tc.tile_pool(name="psS", bufs=2, space="PSUM") as psS,
                tc.tile_pool(name="psY", bufs=1, space="PSUM") as psY,
            ):
                for hl in range(HPC):
                    hp, hrow = hl // 2, (hl % 2) * 64
                    yps = psY.tile([128, T], FP32, tag="y", name="yps")
                    for kt in range(TT):
                        klo = 128 * kt
                        bc0 = klo // BCH
                        at = atp.tile([128, T], FP32R, tag="at", bufs=3, name="at")
                        sums = sm.tile([128, NBC], FP32, tag="sums", bufs=4, name="sums")
                        # S^T in 1024-wide psum chunks, matmuls in <=512 pieces
                        for bc in range(bc0, NBC):
                            blo = max(BCH * bc, klo)
                            s_ps = psS.tile([128, BCH], FP32, tag="s", bufs=2, name="s_ps")
                            for half in range(2):
                                plo = max(blo, BCH * bc + half * QCH)
                                phi = BCH * bc + (half + 1) * QCH
                                if plo >= phi:
                                    continue
                                nc.tensor.matmul(
                                    s_ps[:, plo - BCH * bc:phi - BCH * bc],
                                    kt_pad[:, hl, klo:klo + 128],
                                    qt_sb[:, hp, plo:phi],
                                    start=True, stop=True,
                                )
                            if bc == bc0:
                                off = klo - BCH * bc
                                nc.vector.tensor_add(
                                    s_ps[:, off:off + 128], s_ps[:, off:off + 128], mask_sb
                                )
                            nc.scalar.activation(
                                at[:, blo:BCH * (bc + 1)],
                                s_ps[:, blo - BCH * bc:],
                                Exp, scale=SCALE, accum_out=sums[:, bc:bc + 1],
                            )
                        stot = sm.tile([128, 1], FP32, tag="stot", bufs=4, name="stot")
                        nc.vector.reduce_sum(stot, sums[:, bc0:NBC], axis=mybir.AxisListType.X)
                        rcp = sm.tile([128, 1], FP32, tag="rcp", bufs=4, name="rcp")
                        nc.vector.reciprocal(rcp, stot)
                        vsp = vspad[kt % 2]
                        nc.vector.tensor_scalar_mul(
                            vsp[:, 0:64], v_sb[:, kt, hl * 64:(hl + 1) * 64], rcp
                        )
                        for qc in range(kt // 4, NQC):
                            lo = max(QCH * qc, klo)
                            hi = QCH * qc + QCH
                            nc.tensor.matmul(
                                yps[:, lo:hi], vsp, at[:, lo:hi],
                                start=(kt == 0), stop=(kt == min(TT - 1, 4 * qc + 3)),
                            )
                    nc.vector.tensor_copy(y_sb[hrow:hrow + 64, hp, :], yps[0:64, :])

            # ---- output projection ----
            with tc.tile_pool(name="psP", bufs=4, space="PSUM") as psP:
                for tt in range(TT):
                    po1 = psP.tile([128, QCH], FP32, tag="ps", bufs=4, name="po1")
                    po2 = psP.tile([128, QCH], FP32, tag="ps", bufs=4, name="po2")
                    for et in range(ET):
                        nc.tensor.matmul(
                            po1, y_sb[:, et, tt * 128:(tt + 1) * 128], wp_sb[:, et, 0:QCH],
                            start=(et == 0), stop=(et == ET - 1),
                        )
                        nc.tensor.matmul(
                            po2[:, :C - QCH], y_sb[:, et, tt * 128:(tt + 1) * 128],
                            wp_sb[:, et, QCH:C],
                            start=(et == 0), stop=(et == ET - 1),
                        )
                    o_sb = op.tile([128, C], FP32, tag="o", bufs=3, name="o_sb")
                    nc.vector.tensor_copy(o_sb[:, 0:QCH], po1)
                    nc.vector.tensor_copy(o_sb[:, QCH:C], po2[:, :C - QCH])
                    nc.sync.dma_start(out=out[tt * 128:(tt + 1) * 128, :], in_=o_sb)

    _split_sync_waits(nc)
    return nc


_nc_cache = {}
last_result = None


def kernel(x, w_attn, b_attn, w_proj, b_proj):
    global last_result
    if "nc" not in _nc_cache:
        _nc_cache["nc"] = _build()
    nc = _nc_cache["nc"]

    x = np.asarray(x, dtype=np.float32)
    w_attn = np.asarray(w_attn, dtype=np.float32)
    b_attn = np.asarray(b_attn, dtype=np.float32)
    w_proj = np.asarray(w_proj, dtype=np.float32)
    b_proj = np.asarray(b_proj, dtype=np.float32)

    tri = np.where(
        np.arange(128)[None, :] >= np.arange(128)[:, None], 0.0, MASKV
    ).astype(np.float32)

    in_maps = []
    for core in range(NCORES):
        b = core // 2
        e0 = (core % 2) * E
        in_maps.append({
            "xT": np.ascontiguousarray(x[b].T),
            "wq": np.ascontiguousarray(w_attn[:, e0:e0 + E]),
            "wk": np.ascontiguousarray(w_attn[:, C + e0:C + e0 + E]),
            "wv": np.ascontiguousarray(w_attn[:, 2 * C + e0:2 * C + e0 + E]),
            "bq": np.ascontiguousarray(b_attn[e0:e0 + E]),
            "bk": np.ascontiguousarray(b_attn[C + e0:C + e0 + E]),
            "bv": np.ascontiguousarray(b_attn[2 * C + e0:2 * C + e0 + E]),
            "wp": np.ascontiguousarray(w_proj[e0:e0 + E, :]),
            "mask": tri,
        })

    trace = os.environ.get("ATT_TRACE", "0")
    kw = {}
    if trace != "0":
        n = min(int(trace), NCORES)
        kw = dict(trace=True, trace_cores=list(range(n)))
    res = run_bass_kernel_spmd(nc, in_maps, list(range(NCORES)), **kw)
    last_result = res

    out = np.zeros((B, T, C), dtype=np.float32)
    for core in range(NCORES):
        out[core // 2] += res.results[core]["out"]
    out += b_proj[None, None, :]
    return out
